# revision 8
# baseline (speedup 1.0000x reference)
"""Trainium2 Bass kernel for ClassForgeEnsembleGNN (SAGE -> GAT -> RGCN).

v2 design (8 NeuronCores, SPMD) — see transcript for derivation:
  - Host bakes all selection structure into streamed operands:
    S1/st1 (fp8 dst-onehots, fwd + transposed), Sr (bf16, 1/cnt_r folded),
    xg_sage (bf16 pre-gathered x[src] * 1/cnt[dst] rows -> SAGE needs no
    device gather).
  - GAT/RGCN edge rows via GPSIMD ap_gather from resident SBUF f32 tables
    (feature-major x1/x2), per-tile TensorE transpose to edge-major.
  - GAT self-loops are ordinary edges in the stream.
  - ev logits via PSUM matmuls; leaky+exp batched per gather group; exp
    folded into gathered rows; both heads aggregated in one matmul with
    ones-column denominators.
  - RGCN relation runs = partition sub-ranges of shared tiles; W_r and
    W_root accumulate in one PSUM.
"""

import sys
import os

for _p in ("/opt/trn_rl_repo", "/root/.axon_site/_ro/trn_rl_repo"):
    if os.path.isdir(_p) and _p not in sys.path:
        sys.path.append(_p)

import numpy as np
import ml_dtypes

import concourse.bacc as bacc
import concourse.mybir as mybir
import concourse.tile as tile
from concourse.bass_utils import run_bass_kernel_spmd
from concourse.masks import make_identity

P = 128
NCORES = 8
N = 50000
E = 400000
D = 128
H = 2
R = 5
NEG = 0.2
SH = N // NCORES            # 6250
B = (SH + P - 1) // P       # 49
HSPLIT = 32768
G = 8                       # tiles per gather/load group

f32 = mybir.dt.float32
bf16 = mybir.dt.bfloat16
i16 = mybir.dt.int16
fp8 = mybir.dt.float8e4
AF = mybir.ActivationFunctionType
ALU = mybir.AluOpType
BF = ml_dtypes.bfloat16
F8 = ml_dtypes.float8_e4m3


def _pack(src_pc, dst_pc, rel_pc, nrel):
    """Uniform (half, block[, rel]) slot layout across cores."""
    counts = np.zeros((NCORES, 2, B, nrel), np.int64)
    for k in range(NCORES):
        h = (src_pc[k] >= HSPLIT).astype(np.int64)
        blk = dst_pc[k] // P
        r = rel_pc[k] if nrel > 1 else np.zeros(len(src_pc[k]), np.int64)
        np.add.at(counts[k], (h, blk, r), 1)
    caps = counts.max(0)
    run_off = np.zeros((2, B, nrel + 1), np.int64)
    np.cumsum(caps, axis=2, out=run_off[:, :, 1:])
    sec_slots = run_off[:, :, nrel]
    sec_tiles = (sec_slots + P - 1) // P
    tile_off = np.zeros((2, B + 1), np.int64)
    np.cumsum(sec_tiles, axis=1, out=tile_off[:, 1:])
    T = (int(tile_off[0, B]), int(tile_off[1, B]))

    slots = []
    for k in range(NCORES):
        h = (src_pc[k] >= HSPLIT).astype(np.int64)
        blk = dst_pc[k] // P
        r = rel_pc[k] if nrel > 1 else np.zeros(len(src_pc[k]), np.int64)
        gid = (h * B + blk) * nrel + r
        order = np.argsort(gid, kind="stable")
        gs = gid[order]
        cnt_flat = counts[k].reshape(-1)
        starts = np.concatenate([[0], np.cumsum(cnt_flat)])[:-1]
        rank = np.arange(len(gs)) - starts[gs]
        hh, rest = gs // (B * nrel), gs % (B * nrel)
        bb, rr = rest // nrel, rest % nrel
        slot = tile_off[hh, bb] * P + run_off[hh, bb, rr] + rank
        slots.append((order, hh, slot))
    return dict(run_off=run_off, sec_tiles=sec_tiles,
                tile_off=tile_off, T=T, slots=slots)


def _idx16(idx_per_slot):
    m = idx_per_slot.reshape(-1, 16).T.astype(np.int16)
    return np.tile(m, (8, 1))


def _preprocess(x, edge_index, edge_type):
    src = edge_index[0].astype(np.int64)
    dst = edge_index[1].astype(np.int64)
    et = edge_type.astype(np.int64)

    cnt = np.bincount(dst, minlength=N).astype(np.float32)
    wrec = 1.0 / np.maximum(cnt, 1.0)
    cnt_r = np.zeros((R, N), np.float32)
    for r in range(R):
        cnt_r[r] = np.bincount(dst[et == r], minlength=N)
    wrrec = 1.0 / np.maximum(cnt_r, 1.0)

    shard_of = dst // SH
    e1_src, e1_dst, e1_loop = [], [], []
    e2_src, e2_dst, e2_rel = [], [], []
    for k in range(NCORES):
        sel = shard_of == k
        ls = np.arange(SH, dtype=np.int64) + k * SH
        e1_src.append(np.concatenate([src[sel], ls]))
        e1_dst.append(np.concatenate([dst[sel] - k * SH,
                                      np.arange(SH, dtype=np.int64)]))
        e1_loop.append(np.concatenate([np.zeros(int(sel.sum()), bool),
                                       np.ones(SH, bool)]))
        e2_src.append(src[sel])
        e2_dst.append(dst[sel] - k * SH)
        e2_rel.append(et[sel])

    L1 = _pack(e1_src, e1_dst, [None] * NCORES, 1)
    L2 = _pack(e2_src, e2_dst, e2_rel, R)

    xf = np.asarray(x, np.float32)
    cores = []
    for k in range(NCORES):
        core = {}
        order, hh, slot = L1["slots"][k]
        s_src = e1_src[k][order]
        s_dst = e1_dst[k][order]
        s_loop = e1_loop[k][order]
        for h in range(2):
            T = L1["T"][h]
            if T == 0:
                continue
            m = hh == h
            sl = slot[m]
            idx = np.zeros(T * P, np.int64)
            idx[sl] = s_src[m] - h * HSPLIT
            core[f"ei{h}"] = _idx16(idx)
            SS = np.zeros((P, T * 2 * P), np.float32)
            tt, ee, nn = sl // P, sl % P, s_dst[m] % P
            SS[ee, tt * 2 * P + nn] = 1.0                 # S1 half
            SS[nn, tt * 2 * P + P + ee] = 1.0             # ST1 half
            core[f"ss{h}"] = SS.astype(F8)
            # SAGE still wants plain S1 tiles (for the xg_sage matmul)
            S1 = np.zeros((P, T * P), np.float32)
            S1[ee, tt * P + nn] = 1.0
            core[f"s1{h}"] = S1.astype(F8)
            mr = m & ~s_loop
            slr = slot[mr]
            rows = (xf[s_src[mr]]
                    * wrec[s_dst[mr] + k * SH][:, None])      # [nr, D]
            XG = np.zeros((P, T * P), np.float32)
            XG[(slr % P)[:, None],
               ((slr // P) * P)[:, None] + np.arange(D)[None, :]] = rows
            core[f"xg{h}"] = XG.astype(BF)

        order, hh, slot = L2["slots"][k]
        s_src = e2_src[k][order]
        s_dst = e2_dst[k][order]
        s_rel = e2_rel[k][order]
        for h in range(2):
            T = L2["T"][h]
            if T == 0:
                continue
            m = hh == h
            sl = slot[m]
            idx = np.zeros(T * P, np.int64)
            idx[sl] = s_src[m] - h * HSPLIT
            core[f"ri{h}"] = _idx16(idx)
        cores.append(core)

    runs_per_block = []
    runid = [dict(), dict()]   # (t, r) -> global run index within half
    for b in range(B):
        runs = []
        for h in range(2):
            t0 = int(L2["tile_off"][h, b])
            ntile = int(L2["sec_tiles"][h, b])
            for tl in range(ntile):
                s0, s1_ = tl * P, (tl + 1) * P
                for r in range(R):
                    lo = int(L2["run_off"][h, b, r])
                    hi = int(L2["run_off"][h, b, r + 1])
                    if lo < s1_ and hi > s0:
                        j = len(runid[h])
                        runid[h][(t0 + tl, r)] = j
                        runs.append((h, t0 + tl, r, j))
        runs_per_block.append(runs)
    NR = (len(runid[0]), len(runid[1]))
    # runs of a tile are consecutive j's; per-tile first run index:
    tile_run0 = [dict(), dict()]
    for h in range(2):
        for (t, r), j in runid[h].items():
            if t not in tile_run0[h] or j < tile_run0[h][t]:
                tile_run0[h][t] = j

    # masked run-major Sr arrays per core
    for k in range(NCORES):
        core = cores[k]
        order, hh, slot = L2["slots"][k]
        s_dst = e2_dst[k][order]
        s_rel = e2_rel[k][order]
        s_src = e2_src[k][order]
        for h in range(2):
            if NR[h] == 0:
                continue
            m = hh == h
            sl = slot[m]
            tt, ee, nn = sl // P, sl % P, s_dst[m] % P
            rr = s_rel[m]
            jj = np.array([runid[h][(int(t_), int(r_))]
                           for t_, r_ in zip(tt, rr)], np.int64)
            SRM = np.zeros((P, NR[h] * P), np.float32)
            SRM[ee, jj * P + nn] = wrrec[rr, s_dst[m] + k * SH]
            core[f"sr{h}"] = SRM.astype(BF)

    return dict(L1=L1, L2=L2, cores=cores, runs=runs_per_block,
                NR=NR, tile_run0=tile_run0)


def _tiles_of_block(L, b):
    return [(h, t) for h in range(2)
            for t in range(int(L["tile_off"][h, b]),
                           int(L["tile_off"][h, b + 1]))]


def _block_of_tile(L):
    bo = {}
    for h in range(2):
        for b in range(B):
            for t in range(int(L["tile_off"][h, b]),
                           int(L["tile_off"][h, b + 1])):
                bo[(h, t)] = b
    return bo


def _build_program(pp):
    L1, L2 = pp["L1"], pp["L2"]
    runs_pb = pp["runs"]
    tile_run0 = pp["tile_run0"]
    bo1 = _block_of_tile(L1)

    nc = bacc.Bacc("TRN2", target_bir_lowering=False, debug=False,
                   num_devices=NCORES, num_swdge_queues=4)

    dts = {}

    def din(name, shape, dtype):
        dts[name] = nc.dram_tensor(name, shape, dtype, kind="ExternalInput")

    for h in range(2):
        if L1["T"][h]:
            T = L1["T"][h]
            din(f"ei{h}", [P, T * 8], i16)
            din(f"s1{h}", [P, T * P], fp8)
            din(f"ss{h}", [P, T * 2 * P], fp8)
            din(f"xg{h}", [P, T * P], bf16)
        if L2["T"][h]:
            din(f"ri{h}", [P, L2["T"][h] * 8], i16)
        if pp["NR"][h]:
            din(f"sr{h}", [P, pp["NR"][h] * P], bf16)
    din("xt", [B, P, P], bf16)
    din("wsl", [D, D], bf16)
    din("wsr", [D, D], bf16)
    din("bs", [P, 1], f32)
    din("vsd", [D, 4], f32)
    din("wg0", [D, D], bf16)
    din("wg1", [D, D], bf16)
    din("bg", [P, 1], f32)
    din("wroot", [D, D], bf16)
    din("wr", [D, R * D], bf16)
    din("br", [P, 1], f32)
    out_dram = nc.dram_tensor("out", [SH, 3 * D], f32, kind="ExternalOutput")

    rg = [list(range(NCORES))]

    with tile.TileContext(nc) as tc:
        with (
            tc.tile_pool(name="const", bufs=1) as cb,
            tc.tile_pool(name="dram", bufs=1, space="DRAM") as dr,
        ):
            identf = cb.tile([P, P], f32)
            make_identity(nc, identf[:])
            identb = cb.tile([P, P], bf16)
            nc.vector.tensor_copy(identb[:], identf[:])

            def load_const(pool, name):
                d_ = dts[name]
                t = pool.tile(list(d_.shape), d_.dtype, name=name + "_sb")
                nc.sync.dma_start(t[:], d_[:])
                return t

            bs_sb = load_const(cb, "bs")
            bg_sb = load_const(cb, "bg")
            br_sb = load_const(cb, "br")
            vsd_sb = load_const(cb, "vsd")
            vsdb = cb.tile([P, 4], bf16)
            nc.vector.tensor_copy(vsdb[:], vsd_sb[:])
            adb_sb = cb.tile([P, 2 * B], bf16)

            cc1_in = dr.tile([SH, 256], bf16)
            cc1_out = dr.tile([N, 256], bf16, addr_space="Shared")
            cc2_in = dr.tile([SH, D], bf16)
            cc2_out = dr.tile([N, D], bf16, addr_space="Shared")
            x2tb_dr = dr.tile([P, B * P], bf16)
            qrr = [0]

            def next_q():
                q = qrr[0]
                qrr[0] = (q + 1) % 4
                return q

            # ================= Stage 1: SAGE =================
            sc = nc.enter_named_scope("sage", False)
            with (
                tc.tile_pool(name="sg", bufs=2) as sg,
                tc.tile_pool(name="ps1", bufs=1, space="PSUM") as ps1,
            ):
                wsl_sb = load_const(sg, "wsl")
                wsr_sb = load_const(sg, "wsr")
                groups = {}

                def sage_group(h, g):
                    if (h, g) not in groups:
                        T = L1["T"][h]
                        g0 = g * G
                        gn = min(G, T - g0)
                        xgs = sg.tile([P, G * P], bf16, tag=f"xgs{h}", bufs=2)
                        nc.sync.dma_start(
                            xgs[:, :gn * P],
                            dts[f"xg{h}"][:, g0 * P:(g0 + gn) * P])
                        s1s = sg.tile([P, G * P], fp8, tag=f"s1s{h}", bufs=2)
                        nc.sync.dma_start(
                            s1s[:, :gn * P],
                            dts[f"s1{h}"][:, g0 * P:(g0 + gn) * P])
                        groups[(h, g)] = (xgs, s1s)
                    return groups[(h, g)]

                for b in range(B):
                    vld = min(P, SH - b * P)
                    r0 = b * P
                    tiles = _tiles_of_block(L1, b)
                    paT = ps1.tile([P, P], f32, tag="accA", bufs=2)
                    for j, (h, t) in enumerate(tiles):
                        xgs, s1s = sage_group(h, t // G)
                        o = (t % G) * P
                        nc.tensor.matmul(paT[:], lhsT=xgs[:, o:o + P],
                                         rhs=s1s[:, o:o + P],
                                         start=(j == 0),
                                         stop=(j == len(tiles) - 1))
                    aggTb = sg.tile([P, P], bf16, tag="aggTb")
                    nc.scalar.copy(aggTb[:], paT[:])
                    xt_b = sg.tile([P, P], bf16, tag="xtb")
                    nc.sync.dma_start(xt_b[:], dts["xt"][b, :, :])
                    px1 = ps1.tile([P, P], f32, tag="mm", bufs=2)
                    nc.tensor.matmul(px1[:], lhsT=wsl_sb[:], rhs=aggTb[:],
                                     start=True, stop=False)
                    nc.tensor.matmul(px1[:], lhsT=wsr_sb[:], rhs=xt_b[:],
                                     start=False, stop=True)
                    x1Tb = sg.tile([P, P], bf16, tag="x1Tb")
                    nc.scalar.activation(x1Tb[:], px1[:], AF.Relu,
                                         bias=bs_sb[:, 0:1], scale=1.0)
                    x1Tf = sg.tile([P, P], f32, tag="x1Tf")
                    nc.vector.tensor_scalar(out=x1Tf[:], in0=px1[:],
                                            scalar1=bs_sb[:, 0:1],
                                            scalar2=0.0, op0=ALU.add,
                                            op1=ALU.max)
                    pad_ = ps1.tile([P, 4], f32, tag="pad", bufs=1)
                    nc.tensor.matmul(pad_[:], lhsT=x1Tb[:], rhs=vsdb[:],
                                     start=True, stop=True)
                    nc.scalar.copy(adb_sb[:, 2 * b:2 * b + 2], pad_[:, 2:4])
                    ase = sg.tile([P, 3], bf16, tag="ase")
                    nc.vector.memset(ase[:, 0:1], 1.0)
                    nc.scalar.copy(ase[:, 1:3], pad_[:, 0:2])
                    ptr = ps1.tile([P, P], f32, tag="mm", bufs=2)
                    nc.tensor.matmul(ptr[:], lhsT=x1Tf[:], rhs=identf[:],
                                     start=True, stop=True)
                    x1n = sg.tile([P, P], f32, tag="x1n")
                    nc.scalar.copy(x1n[:], ptr[:])
                    x1nb = sg.tile([P, P], bf16, tag="x1nb")
                    nc.vector.tensor_copy(x1nb[:], ptr[:])
                    nc.sync.dma_start(out_dram[r0:r0 + vld, 0:D],
                                      x1n[:vld, :])
                    nc.sync.dma_start(cc1_in[r0:r0 + vld, 0:D],
                                      x1nb[:vld, :])
                    nc.sync.dma_start(cc1_in[r0:r0 + vld, D:D + 3],
                                      ase[:vld, :])
            nc.leave_named_scope("sage", sc[0], False)

            sc = nc.enter_named_scope("ag1", False)
            nc.gpsimd.collective_compute(
                "AllGather", ALU.bypass, replica_groups=rg,
                ins=[cc1_in[:]], outs=[cc1_out[:]])
            nc.leave_named_scope("ag1", sc[0], False)

            # ================= Stage 2: GAT =================
            sc = nc.enter_named_scope("gat", False)
            with (
                tc.tile_pool(name="g2", bufs=2) as g2,
                tc.tile_pool(name="ps2", bufs=1, space="PSUM") as ps2,
            ):
                wg0_sb = load_const(g2, "wg0")
                wg1_sb = load_const(g2, "wg1")
                tviews = (cc1_out[0:HSPLIT, :], cc1_out[HSPLIT:N, :])
                eidx = {}
                for h in range(2):
                    if L1["T"][h]:
                        t_ = g2.tile([P, L1["T"][h] * 8], i16,
                                     name=f"eidx{h}")
                        nc.sync.dma_start(t_[:], dts[f"ei{h}"][:])
                        eidx[h] = t_

                groups = {}

                def gat_group(h, g):
                    if (h, g) in groups:
                        return groups[(h, g)]
                    T = L1["T"][h]
                    g0 = g * G
                    gn = min(G, T - g0)
                    xgg = g2.tile([P, G, 256], bf16, tag=f"xgg{h}", bufs=4)
                    nc.gpsimd.dma_gather(
                        xgg[:, 0:gn, :], tviews[h],
                        eidx[h][:, g0 * 8:(g0 + gn) * 8],
                        gn * P, gn * P, 256, queue_num=next_q())
                    ss = g2.tile([P, G, 2 * P], fp8, tag=f"ssg{h}", bufs=2)
                    nc.sync.dma_start(
                        ss[:, 0:gn, :],
                        dts[f"ss{h}"][:, g0 * 2 * P:(g0 + gn) * 2 * P])
                    evs = g2.tile([P, G, 2], f32, tag=f"evs{h}")
                    for ti in range(gn):
                        bt = bo1[(h, g0 + ti)]
                        evB = ps2.tile([P, 2], f32, tag=f"evB{h}", bufs=1)
                        nc.tensor.matmul(
                            evB[:],
                            lhsT=ss[:, ti, P:2 * P],
                            rhs=adb_sb[:, 2 * bt:2 * bt + 2],
                            start=True, stop=True)
                        nc.vector.tensor_tensor(
                            out=evs[:, ti, :],
                            in0=xgg[:, ti, 129:131],
                            in1=evB[:], op=ALU.add)
                    ev2 = g2.tile([P, G, 2], f32, tag=f"ev2{h}")
                    nc.vector.tensor_scalar(out=ev2[:, :gn, :],
                                            in0=evs[:, :gn, :],
                                            scalar1=NEG, scalar2=None,
                                            op0=ALU.mult)
                    ev3 = g2.tile([P, G, 2], f32, tag=f"ev3{h}")
                    nc.vector.tensor_tensor(out=ev3[:, :gn, :],
                                            in0=ev2[:, :gn, :],
                                            in1=evs[:, :gn, :], op=ALU.max)
                    exg = g2.tile([P, G, 2], f32, tag=f"exg{h}")
                    nc.scalar.activation(exg[:, :gn, :], ev3[:, :gn, :],
                                         AF.Exp)
                    groups[(h, g)] = (xgg, ss, exg)
                    return groups[(h, g)]

                for b in range(B):
                    vld = min(P, SH - b * P)
                    r0 = b * P
                    tiles = _tiles_of_block(L1, b)
                    p01 = ps2.tile([P, 258], f32, tag="accA", bufs=2)
                    for j, (h, t) in enumerate(tiles):
                        g, ti = t // G, t % G
                        xgg, ss, exg = gat_group(h, g)
                        xgp = g2.tile([P, 258], bf16, tag="xgp", bufs=3)
                        nc.vector.tensor_scalar(
                            out=xgp[:, 0:129], in0=xgg[:, ti, 0:129],
                            scalar1=exg[:, ti, 0:1],
                            scalar2=None, op0=ALU.mult)
                        nc.vector.tensor_scalar(
                            out=xgp[:, 129:258], in0=xgg[:, ti, 0:129],
                            scalar1=exg[:, ti, 1:2],
                            scalar2=None, op0=ALU.mult)
                        nc.tensor.matmul(p01[:],
                                         lhsT=ss[:, ti, 0:P],
                                         rhs=xgp[:],
                                         start=(j == 0),
                                         stop=(j == len(tiles) - 1))
                    den = g2.tile([P, 2], f32, tag="den")
                    nc.vector.tensor_copy(den[:, 0:1], p01[:, 128:129])
                    nc.vector.tensor_copy(den[:, 1:2], p01[:, 257:258])
                    den2 = g2.tile([P, 2], f32, tag="den2")
                    nc.vector.tensor_scalar(out=den2[:], in0=den[:],
                                            scalar1=1e-30, scalar2=None,
                                            op0=ALU.max)
                    rec = g2.tile([P, 2], f32, tag="rec")
                    nc.vector.reciprocal(rec[:], den2[:])
                    nm0 = g2.tile([P, P], bf16, tag="nm0")
                    nc.vector.tensor_scalar(out=nm0[:], in0=p01[:, 0:128],
                                            scalar1=rec[:, 0:1],
                                            scalar2=None, op0=ALU.mult)
                    nm1 = g2.tile([P, P], bf16, tag="nm1")
                    nc.vector.tensor_scalar(out=nm1[:], in0=p01[:, 129:257],
                                            scalar1=rec[:, 1:2],
                                            scalar2=None, op0=ALU.mult)
                    ptm0 = ps2.tile([P, P], f32, tag="mmT", bufs=2)
                    nc.tensor.matmul(ptm0[:], lhsT=nm0[:], rhs=identb[:],
                                     start=True, stop=True)
                    nm0T = g2.tile([P, P], bf16, tag="nm0T")
                    nc.scalar.copy(nm0T[:], ptm0[:])
                    ptm1 = ps2.tile([P, P], f32, tag="mmT", bufs=2)
                    nc.tensor.matmul(ptm1[:], lhsT=nm1[:], rhs=identb[:],
                                     start=True, stop=True)
                    nm1T = g2.tile([P, P], bf16, tag="nm1T")
                    nc.vector.tensor_copy(nm1T[:], ptm1[:])
                    x2m = ps2.tile([P, P], f32, tag="x2m", bufs=1)
                    nc.tensor.matmul(x2m[:], lhsT=wg0_sb[:], rhs=nm0T[:],
                                     start=True, stop=False)
                    nc.tensor.matmul(x2m[:], lhsT=wg1_sb[:], rhs=nm1T[:],
                                     start=False, stop=True)
                    x2Tb = g2.tile([P, P], bf16, tag="x2Tb")
                    nc.scalar.activation(x2Tb[:], x2m[:], AF.Relu,
                                         bias=bg_sb[:, 0:1], scale=0.5)
                    x2Tf = g2.tile([P, P], f32, tag="x2Tf")
                    nc.vector.tensor_scalar(out=x2Tf[:], in0=x2m[:],
                                            scalar1=0.5,
                                            scalar2=bg_sb[:, 0:1],
                                            op0=ALU.mult, op1=ALU.add)
                    x2Tf2 = g2.tile([P, P], f32, tag="x2Tf2")
                    nc.vector.tensor_scalar(out=x2Tf2[:], in0=x2Tf[:],
                                            scalar1=0.0, scalar2=None,
                                            op0=ALU.max)
                    nc.sync.dma_start(x2tb_dr[:, r0:r0 + P], x2Tb[:])
                    ptn = ps2.tile([P, P], f32, tag="mmT", bufs=2)
                    nc.tensor.matmul(ptn[:], lhsT=x2Tf2[:], rhs=identf[:],
                                     start=True, stop=True)
                    x2n = g2.tile([P, P], f32, tag="x2n")
                    nc.scalar.copy(x2n[:], ptn[:])
                    x2nb = g2.tile([P, P], bf16, tag="x2nb")
                    nc.vector.tensor_copy(x2nb[:], ptn[:])
                    nc.sync.dma_start(cc2_in[r0:r0 + vld, :], x2nb[:vld, :])
                    nc.sync.dma_start(out_dram[r0:r0 + vld, D:2 * D],
                                      x2n[:vld, :])
            nc.leave_named_scope("gat", sc[0], False)

            sc = nc.enter_named_scope("ag2", False)
            nc.gpsimd.collective_compute(
                "AllGather", ALU.bypass, replica_groups=rg,
                ins=[cc2_in[:]], outs=[cc2_out[:]])
            nc.leave_named_scope("ag2", sc[0], False)

            # ================= Stage 3: RGCN =================
            sc = nc.enter_named_scope("rgcn", False)
            with (
                tc.tile_pool(name="g3", bufs=2) as g3,
                tc.tile_pool(name="ps3", bufs=1, space="PSUM") as ps3,
            ):
                wroot_sb = load_const(g3, "wroot")
                wr_sb = load_const(g3, "wr")
                tviews2 = (cc2_out[0:HSPLIT, :], cc2_out[HSPLIT:N, :])
                ridx = {}
                for h in range(2):
                    if L2["T"][h]:
                        t_ = g3.tile([P, L2["T"][h] * 8], i16,
                                     name=f"ridx{h}")
                        nc.sync.dma_start(t_[:], dts[f"ri{h}"][:])
                        ridx[h] = t_

                groups = {}

                MAXRG = 5 * G   # run-tiles per group upper bound

                def rg_group(h, g):
                    if (h, g) in groups:
                        return groups[(h, g)]
                    T = L2["T"][h]
                    g0 = g * G
                    gn = min(G, T - g0)
                    xgg = g3.tile([P, G, D], bf16, tag=f"xgg{h}", bufs=4)
                    nc.gpsimd.dma_gather(
                        xgg[:, 0:gn, :], tviews2[h],
                        ridx[h][:, g0 * 8:(g0 + gn) * 8],
                        gn * P, gn * P, D, queue_num=next_q())
                    j0 = tile_run0[h][g0]
                    j1 = (tile_run0[h][g0 + gn] if (g0 + gn) in tile_run0[h]
                          else pp["NR"][h])
                    nr = j1 - j0
                    assert nr <= MAXRG, (h, g, nr)
                    sr = g3.tile([P, MAXRG * P], bf16, tag=f"srg{h}", bufs=2)
                    nc.sync.dma_start(
                        sr[:, :nr * P],
                        dts[f"sr{h}"][:, j0 * P:j1 * P])
                    groups[(h, g)] = (xgg, sr, j0)
                    return groups[(h, g)]

                RTAG = ["pr0", "pr1", "pr2", "pr3", "pr4"]
                for b in range(B):
                    vld = min(P, SH - b * P)
                    r0 = b * P
                    runs = runs_pb[b]
                    tiles = _tiles_of_block(L2, b)
                    by_tile = {}
                    for (h, t, r, j) in runs:
                        by_tile.setdefault((h, t), []).append((r, j))
                    present = sorted({r for (_, _, r, _) in runs})
                    first = {r: True for r in present}
                    lastof = {}
                    for (h, t, r, j) in runs:
                        lastof[r] = j
                    prs = {}
                    for jt, (h, t) in enumerate(tiles):
                        g, ti = t // G, t % G
                        xgg, sr, jg0 = rg_group(h, g)
                        for (r, j) in by_tile.get((h, t), []):
                            if r not in prs:
                                prs[r] = ps3.tile(
                                    [P, P], f32, name=f"pr{r}",
                                    tag=RTAG[present.index(r)], bufs=1)
                            nc.tensor.matmul(
                                prs[r][:], lhsT=xgg[:, ti, :],
                                rhs=sr[:, (j - jg0) * P:(j - jg0 + 1) * P],
                                start=first[r], stop=(lastof[r] == j))
                            first[r] = False
                    x2tb = g3.tile([P, P], bf16, tag="x2tb", bufs=2)
                    nc.sync.dma_start(x2tb[:], x2tb_dr[:, r0:r0 + P])
                    px3 = ps3.tile([P, P], f32, tag="mm3", bufs=1)
                    nc.tensor.matmul(px3[:], lhsT=wroot_sb[:], rhs=x2tb[:],
                                     start=True, stop=(len(present) == 0))
                    for i, r in enumerate(present):
                        aggb = g3.tile([P, P], bf16, tag="aggb", bufs=2)
                        if i % 2 == 0:
                            nc.scalar.copy(aggb[:], prs[r][:])
                        else:
                            nc.vector.tensor_copy(aggb[:], prs[r][:])
                        nc.tensor.matmul(px3[:],
                                         lhsT=wr_sb[:, r * D:(r + 1) * D],
                                         rhs=aggb[:], start=False,
                                         stop=(i == len(present) - 1))
                    x3Tf = g3.tile([P, P], f32, tag="x3Tf")
                    nc.vector.tensor_scalar(out=x3Tf[:], in0=px3[:],
                                            scalar1=br_sb[:, 0:1],
                                            scalar2=None, op0=ALU.add)
                    ptn3 = ps3.tile([P, P], f32, tag="mmT", bufs=1)
                    nc.tensor.matmul(ptn3[:], lhsT=x3Tf[:], rhs=identf[:],
                                     start=True, stop=True)
                    x3n = g3.tile([P, P], f32, tag="x3n")
                    nc.scalar.copy(x3n[:], ptn3[:])
                    nc.sync.dma_start(out_dram[r0:r0 + vld, 2 * D:3 * D],
                                      x3n[:vld, :])
            nc.leave_named_scope("rgcn", sc[0], False)

    nc.compile()
    return nc


def kernel(x, edge_index, edge_type, W_sage_l, b_sage, W_sage_r,
           W_gat, att_src, att_dst, b_gat, W_rgcn, W_root, b_rgcn,
           _trace=False, _tmpdir=None):
    x = np.asarray(x, np.float32)
    edge_index = np.asarray(edge_index)
    edge_type = np.asarray(edge_type)

    pp = _preprocess(x, edge_index, edge_type)
    nc = _build_program(pp)

    W_gat = np.asarray(W_gat, np.float32)
    v = np.empty((D, 4), np.float32)
    for h in range(H):
        v[:, h] = W_gat[:, h, :] @ np.asarray(att_src, np.float32)[h]
        v[:, 2 + h] = W_gat[:, h, :] @ np.asarray(att_dst, np.float32)[h]

    common = {
        "wsl": np.asarray(W_sage_l, np.float32).astype(BF),
        "wsr": np.asarray(W_sage_r, np.float32).astype(BF),
        "bs": np.asarray(b_sage, np.float32).reshape(P, 1),
        "vsd": v,
        "wg0": W_gat[:, 0, :].astype(BF),
        "wg1": W_gat[:, 1, :].astype(BF),
        "bg": np.asarray(b_gat, np.float32).reshape(P, 1),
        "wroot": np.asarray(W_root, np.float32).astype(BF),
        "wr": np.ascontiguousarray(
            np.asarray(W_rgcn, np.float32).transpose(1, 0, 2)
            .reshape(D, R * D)).astype(BF),
        "br": np.asarray(b_rgcn, np.float32).reshape(P, 1),
    }

    in_maps = []
    for k in range(NCORES):
        xs = np.zeros((B * P, D), np.float32)
        xs[:SH] = x[k * SH:(k + 1) * SH]
        m = dict(common)
        m["xt"] = np.ascontiguousarray(
            xs.reshape(B, P, D).transpose(0, 2, 1)).astype(BF)
        m.update(pp["cores"][k])
        in_maps.append(m)

    res = run_bass_kernel_spmd(nc, in_maps, core_ids=list(range(NCORES)),
                               trace=_trace, tmpdir=_tmpdir)
    out = np.concatenate([res.results[k]["out"] for k in range(NCORES)], 0)
    if _trace:
        return out, res
    return out
